# revision 63
# baseline (speedup 1.0000x reference)
"""Bass/Trainium2 kernel v3 for nn_BlockForNormalWindow (windowed-attention
transformer block), data-parallel over batch across 8 NeuronCores.

v3 over v2: fp8e4 DoubleRow matmuls for qkv/v/proj/fc1/fc2 (weights x64,
biases folded via ones-row in a 4th K-chunk), window-major fp8 hT
(contiguous group slices, no hstage), bf16 score path with K=92 layout
(no garbage rows), fp8 ets/v with DoubleRow AV, PE-broadcast of 1/z,
Quake rsqrt on DVE for LN2 (no act-table thrash), 3-way engine rotation
for PSUM->SBUF copies, split E1/E2 MLP phase."""
import sys
sys.path.insert(0, '/opt/trn_rl_repo')

import numpy as np
import ml_dtypes
import concourse.bass as bass
import concourse.mybir as mybir
import concourse.tile as tile
from concourse import bacc
from concourse.bass_utils import run_bass_kernel_spmd
from concourse.masks import make_identity

F32 = mybir.dt.float32
I32 = mybir.dt.int32
BF = mybir.dt.bfloat16
F8 = mybir.dt.float8e4
AF = mybir.ActivationFunctionType
ALU = mybir.AluOpType
DR = mybir.MatmulPerfMode.DoubleRow

B, H, W = 8, 64, 64
DIM, NH, WS = 384, 6, 14
HD = DIM // NH
MLP = 4 * DIM
EPS = 1e-5
SCALE = HD ** -0.5
HP = 70
NWIN = 25
NTOK = NWIN * WS * WS        # 4900
NVAL = H * W                 # 4096
VS = 65                      # per-head stride in v layout (64 vals + ones col)
KR = 110                     # rows in k/q operand: q/k 0:64, relh 64:78, relw 96:110
FS = 64.0                    # fp8 weight pre-scale
FSI = 1.0 / FS
QMAGIC = 1.3211836172961055e+19   # 0x5f3759df as float32

GROUPS = [(g * 392, 392) for g in range(12)] + [(4704, 196)]
# E1 group g (image rows 8g:8g+8) ready after this D group index
E1_AFTER = {2: [0], 4: [1, 2], 7: [3, 4], 9: [5, 6], 12: [7]}


def _ap(t, offset_elems, dims, p=None):
    a = t[:, 0:1] if p is None else t[p[0]:p[1], 0:1]
    return bass.AP(tensor=a.tensor, offset=a.offset + offset_elems,
                   ap=[a.ap[0]] + dims)


def _dram_ap(t, offset_elems, dims):
    a = t.ap()
    return bass.AP(tensor=a.tensor, offset=offset_elems, ap=dims)


def build_bass():
    nc = bacc.Bacc("TRN2", target_bir_lowering=False, debug=False)

    x_in = nc.dram_tensor("x", [NVAL, DIM], F32, kind="ExternalInput")
    wqk_in = nc.dram_tensor("wqk", [128, 4 * 2 * DIM], F8, kind="ExternalInput")
    wv_in = nc.dram_tensor("wv", [128, 4 * DIM], F8, kind="ExternalInput")
    rel_in = nc.dram_tensor("rel", [HD, 2 * 196], BF, kind="ExternalInput")
    kpat_in = nc.dram_tensor("kpat", [46, 392], BF, kind="ExternalInput")
    wp_in = nc.dram_tensor("wp", [128, 4 * DIM], F8, kind="ExternalInput")
    w1_in = nc.dram_tensor("w1", [128, 4 * MLP], F8, kind="ExternalInput")
    w2_in = nc.dram_tensor("w2", [128, 12 * DIM], F8, kind="ExternalInput")
    b2_in = nc.dram_tensor("b2", [DIM], F8, kind="ExternalInput")
    out_d = nc.dram_tensor("out", [NVAL, DIM], F32, kind="ExternalOutput")

    # k operand in DRAM: rows 0:64 x [NH, NTOK] bf16, written in phase B.
    kT_d = nc.dram_tensor("kT_d", [64, NH * NTOK], BF)
    y_d = nc.dram_tensor("y_d", [HP * HP, DIM], BF)

    with tile.TileContext(nc) as tc:
      with tc.tile_pool(name="singles", bufs=1) as singles:
        ident_f = singles.tile([128, 128], F32)
        make_identity(nc, ident_f[:])
        identB = singles.tile([128, 128], BF)
        nc.vector.tensor_copy(out=identB[:], in_=ident_f[:])

        eps_t = singles.tile([128, 1], F32)
        nc.vector.memset(eps_t[:], EPS)
        cb64 = singles.tile([1, 64], BF)
        nc.gpsimd.memset(cb64[:], 1.0)
        ones_f8 = singles.tile([1, 128], F8)
        nc.gpsimd.memset(ones_f8[:], 1.0)
        onesrow_f = singles.tile([1, 392], F32)
        nc.gpsimd.memset(onesrow_f[:], 1.0)
        magic_t = singles.tile([128, 4], F32)
        nc.vector.memset(magic_t[:], QMAGIC)
        c15_t = singles.tile([128, 4], F32)
        nc.vector.memset(c15_t[:], 1.5)
        dmy_t = singles.tile([1, 1], F32)

        # weights
        wqk_t = singles.tile([128, 4, 2 * DIM], F8)
        nc.sync.dma_start(out=wqk_t[:], in_=wqk_in.ap())
        wv_t = singles.tile([128, 4, DIM], F8)
        nc.sync.dma_start(out=wv_t[:], in_=wv_in.ap())
        relm_t = singles.tile([HD, 2 * 196], BF)
        nc.sync.dma_start(out=relm_t[:], in_=rel_in.ap())
        wp_t = singles.tile([128, 4, DIM], F8)
        nc.sync.dma_start(out=wp_t[:], in_=wp_in.ap())
        w1_t = singles.tile([128, 4, MLP], F8)
        nc.sync.dma_start(out=w1_t[:], in_=w1_in.ap())
        w2_t = singles.tile([128, 12, DIM], F8)
        nc.sync.dma_start(out=w2_t[:], in_=w2_in.ap())
        b2row = singles.tile([1, DIM], F8)
        nc.sync.dma_start(out=b2row[:], in_=b2_in.ap())

        with tc.tile_pool(name="attops", bufs=1) as attops:
          qb = [attops.tile([KR, NTOK], BF, name=f"qb{h}") for h in range(NH)]
          for h in range(NH):
              # zero the pattern-selected rows incl. never-written 78:96 band
              e = [nc.scalar, nc.gpsimd, nc.scalar,
                   nc.vector, nc.scalar, nc.scalar][h]
              if e is nc.scalar:
                  e.memzero(qb[h][64:96, :])
              else:
                  e.memset(qb[h][64:96, :], 0.0)
          vw = [attops.tile([98, 2, 416], F8, name=f"vw{w}") for w in range(NWIN)]
          for w in range(NWIN):
              e = [nc.vector, nc.gpsimd][w % 2]
              e.memset(_ap(vw[w], 64, [[416, 2], [VS, NH], [1, 1]]), 1.0)
          kta2 = [attops.tile([KR, NH, 392], BF, name=f"kta{i}") for i in range(3)]
          for i in range(3):
              nc.sync.dma_start(
                  out=kta2[i][64:KR, :, :],
                  in_=bass.AP(tensor=kpat_in.ap().tensor, offset=0,
                              ap=[[392, 46], [0, NH], [1, 392]]))

          with tc.tile_pool(name="pHT", bufs=1) as pHT:
            # window-block-interleaved: [(w, k, c)] -> w*896 + k*224 + c
            # (DoubleRow ldweights needs 32-elem-aligned k-pair strides)
            hT = pHT.tile([128, 25, 4, 224], F8, name="hT")
            # chunk 3: row0 = 1.0 (bias row), rows 1:128 = 0
            for i in range(4):
                e = [nc.vector, nc.gpsimd][i % 2]
                dims = [[896, 7], [1, 196]]
                o = 3 * 224 + (7 * i) * 896
                if i == 3:
                    dims = [[896, 4], [1, 196]]
                nc.scalar.memzero(_ap(hT, o, dims, p=(0, 128)))
                e.memset(_ap(hT, o, dims, p=(0, 1)), 1.0)
            # zero padding tokens in chunks 0:3 (right-edge and bottom windows)
            for c in range(3):
                e = [nc.vector, nc.gpsimd][c % 2]
                # right-edge windows w%5==4, cols 8:14 of each window row
                e.memset(_ap(hT, 4 * 896 + c * 224 + 8,
                             [[4480, 5], [14, 14], [1, 6]]), 0.0)
                # bottom windows 20..24, rows 8:14
                e.memset(_ap(hT, 20 * 896 + c * 224 + 8 * 14,
                             [[896, 5], [1, 84]]), 0.0)

            # ===== Phase A: LN1 + transpose into window-major fp8 hT =====
            with tc.tile_pool(name="pA", bufs=4) as pA, \
                 tc.tile_pool(name="pA_ps", bufs=4, space="PSUM") as pA_ps:
                for ch in range(8):
                    xc = pA.tile([128, 4, DIM], F32, tag="xc")
                    nc.sync.dma_start(
                        out=xc[:],
                        in_=_dram_ap(x_in, 512 * ch * DIM,
                                     [[DIM, 128], [128 * DIM, 4], [1, DIM]]))
                    mvall = pA.tile([128, 4, 2], F32, tag="mva")
                    for tt in range(4):
                        stats = pA.tile([128, 6], F32, tag="st")
                        nc.vector.bn_stats(out=stats[:], in_=xc[:, tt, :])
                        nc.vector.bn_aggr(out=mvall[:, tt, :], in_=stats[:])
                    rstd = pA.tile([128, 4], F32, tag="rstd")
                    nc.scalar.activation(out=rstd[:], in_=_ap(mvall, 1, [[2, 4]]),
                                         func=AF.Sqrt, bias=eps_t[:], scale=1.0)
                    nc.vector.reciprocal(out=rstd[:], in_=rstd[:])
                    for tt in range(4):
                        t = 4 * ch + tt
                        nmr = pA.tile([128, 1], F32, tag="nmr")
                        nc.vector.scalar_tensor_tensor(out=nmr[:], in0=mvall[:, tt, 0:1],
                                                       scalar=-1.0, in1=rstd[:, tt:tt + 1],
                                                       op0=ALU.mult, op1=ALU.mult)
                        hn = pA.tile([128, DIM], BF, tag="hn")
                        if tt % 2 == 0:
                            nc.gpsimd.tensor_scalar(out=hn[:], in0=xc[:, tt, :],
                                                    scalar1=nmr[:],
                                                    scalar2=rstd[:, tt:tt + 1],
                                                    op0=ALU.add, op1=ALU.mult)
                        else:
                            nc.scalar.activation(out=hn[:], in_=xc[:, tt, :],
                                                 func=AF.Identity, bias=nmr[:],
                                                 scale=rstd[:, tt:tt + 1])
                        pt = pA_ps.tile([128, 3, 128], BF, tag="tr")
                        for c in range(3):
                            nc.tensor.transpose(pt[:, c, :], hn[:, c * 128:(c + 1) * 128],
                                                identB[:])
                        # window-major scatter: rows r0, r0+1 of the image
                        # (4 full col-windows of 14 + 1 partial of 8)
                        r0 = 2 * t
                        band, rb = r0 // 14, r0 % 14
                        for r in range(2):
                            dst = _ap(hT, band * 4480 + (rb + r) * 14,
                                      [[224, 3], [896, 4], [1, 14]])
                            src = bass.AP(tensor=pt[:].tensor,
                                          offset=pt[:].offset + 64 * r,
                                          ap=[pt[:].ap[0], [128, 3], [1, 56]])
                            if (t + r) % 3 != 2:
                                nc.scalar.copy(out=dst, in_=src)
                            else:
                                nc.vector.tensor_copy(out=dst, in_=src)
                            dst = _ap(hT, band * 4480 + 4 * 896 + (rb + r) * 14,
                                      [[224, 3], [1, 8]])
                            src = bass.AP(tensor=pt[:].tensor,
                                          offset=pt[:].offset + 64 * r + 56,
                                          ap=[pt[:].ap[0], [128, 3], [1, 8]])
                            nc.vector.tensor_copy(out=dst, in_=src)

            # ===== Phase B: qk DR matmuls, rel, v =====
            with tc.tile_pool(name="pB", bufs=3) as pB, \
                 tc.tile_pool(name="pB_ps", bufs=2, space="PSUM") as pB_ps, \
                 tc.tile_pool(name="pC_ps", bufs=2, space="PSUM") as pC_ps, \
                 tc.tile_pool(name="pBv_ps", bufs=2, space="PSUM") as pBv_ps:
                copy_rr = [0]

                def scaled_copy(dst, src):
                    i = copy_rr[0]; copy_rr[0] += 1
                    if i % 2 == 0:
                        nc.scalar.activation(out=dst, in_=src, func=AF.Identity,
                                             bias=0.0, scale=FSI)
                    else:
                        nc.vector.tensor_scalar(out=dst, in0=src, scalar1=FSI,
                                                scalar2=None, op0=ALU.mult)

                def emit_v(s_):
                    w, half = s_ // 2, s_ % 2
                    ps = pBv_ps.tile([98, DIM], F32, tag="v", name="vps")
                    for j in range(2):
                        nc.tensor.matmul(ps[:],
                                         _ap(hT, w * 896 + 2 * j * 224 + 98 * half,
                                             [[224, 2], [1, 98]]),
                                         wv_t[:, 2 * j:2 * j + 2, :],
                                         start=(j == 0), stop=(j == 1), perf_mode=DR)
                    scaled_copy(_ap(vw[w], half * 416, [[VS, NH], [1, 64]]), ps[:])

                vnext = [0]
                relc = [0]
                pending_rel = []

                def rel_pack(h, r0, n, is_row):
                    # n rel matmuls into one 4-bank psum tile, one big copy out
                    def go():
                        ps = pC_ps.tile([14, 2, 512], F32, tag="rel")
                        for k in range(n):
                            r = r0 + k
                            if is_row:
                                nc.tensor.matmul(
                                    ps[:, k, 0:350],
                                    relm_t[:, r * 14:(r + 1) * 14],
                                    _ap(qb[h], r * 14, [[196, 25], [1, 14]],
                                        p=(0, 64)),
                                    start=True, stop=True)
                            else:
                                nc.tensor.matmul(
                                    ps[:, k, 0:350],
                                    relm_t[:, 196 + r * 14:196 + (r + 1) * 14],
                                    _ap(qb[h], r, [[196, 25], [14, 14]],
                                        p=(0, 64)),
                                    start=True, stop=True)
                        src = _ap(ps, 0, [[512, n], [1, 350]], p=(0, 14))
                        if is_row:
                            dst = _ap(qb[h], r0 * 14,
                                      [[14, n], [196, 25], [1, 14]], p=(64, 78))
                        else:
                            dst = _ap(qb[h], r0,
                                      [[1, n], [196, 25], [14, 14]], p=(96, 110))
                        i = relc[0]; relc[0] += 1
                        if i % 2 == 0:
                            nc.scalar.copy(out=dst, in_=src)
                        else:
                            nc.vector.tensor_copy(out=dst, in_=src)
                    return go

                for m in range(6):
                    if m == 5:
                        # preload the Exp act table during B's tail so phase
                        # D's first softmax doesn't pay the 1.3us load
                        nc.scalar.activation(out=dmy_t[:], in_=eps_t[0:1, 0:1],
                                             func=AF.Exp, bias=0.0, scale=1.0)
                    for gi, (p0, plen) in enumerate(GROUPS):
                        ps = pB_ps.tile([128, 392], F32, tag="qk")
                        w0 = p0 // 196
                        nwing = plen // 196
                        for j in range(2):
                            nc.tensor.matmul(
                                ps[:, 0:plen],
                                wqk_t[:, 2 * j:2 * j + 2, m * 128:(m + 1) * 128],
                                _ap(hT, w0 * 896 + 2 * j * 224,
                                    [[224, 2], [896, nwing], [1, 196]]),
                                start=(j == 0), stop=(j == 1), perf_mode=DR)
                        if m < 3:
                            for half in range(2):
                                h = 2 * m + half
                                scaled_copy(qb[h][0:64, p0:p0 + plen],
                                            ps[64 * half:64 * half + 64, 0:plen])
                        else:
                            mm = m - 3
                            kst = pB.tile([128, 392], BF, tag="kst", bufs=4,
                                          name="kst")
                            scaled_copy(kst[:, 0:plen], ps[:, 0:plen])
                            nc.sync.dma_start(
                                out=_dram_ap(kT_d, 2 * mm * NTOK + p0,
                                             [[NTOK, 2], [NH * NTOK, 64], [1, plen]]),
                                in_=kst[:, 0:plen])
                        if vnext[0] < 50 and (m, gi) != (0, 0):
                            emit_v(vnext[0])
                            vnext[0] += 1
                        for _ in range(3):
                            if pending_rel:
                                pending_rel.pop(0)()
                    if m < 3:
                        # queue rel rows for heads 2m, 2m+1 (q complete now);
                        # they interleave into the next m's group loop
                        for r0, n in ((0, 2), (2, 2), (4, 2), (6, 2), (8, 2), (10, 2), (12, 2)):
                            for half in range(2):
                                h = 2 * m + half
                                pending_rel.append(rel_pack(h, r0, n, True))
                                pending_rel.append(rel_pack(h, r0, n, False))
                while pending_rel:
                    pending_rel.pop(0)()

          # ===== Phase D: attention + proj (hT freed) — with E1 interleaved ===
          with tc.tile_pool(name="pE1p", bufs=1) as pE1p:
            zts = [pE1p.tile([128, DIM], BF, name=f"zts{t}") for t in range(32)]
            hns = [pE1p.tile([128, DIM], BF, name=f"hns{t}") for t in range(32)]
            mvs = [pE1p.tile([128, 4, 2], F32, name=f"mv{g}") for g in range(8)]

            with tc.tile_pool(name="pD", bufs=4) as pD, \
                 tc.tile_pool(name="pDet", bufs=10) as pDet, \
                 tc.tile_pool(name="pDa", bufs=2) as pDa, \
                 tc.tile_pool(name="pE1", bufs=2) as pE1, \
                 tc.tile_pool(name="pDs_ps", bufs=2, space="PSUM") as pDs_ps, \
                 tc.tile_pool(name="pDo_ps", bufs=2, space="PSUM") as pDo_ps, \
                 tc.tile_pool(name="pDm_ps", bufs=2, space="PSUM") as pDm_ps:

                def e1_group(g):
                    xc = pE1.tile([128, 4, DIM], F32, tag="xe", name="xc")
                    nc.sync.dma_start(
                        out=xc[:],
                        in_=_dram_ap(x_in, 512 * g * DIM,
                                     [[DIM, 128], [128 * DIM, 4], [1, DIM]]))
                    mvall = mvs[g]
                    for tt in range(4):
                        yc = pE1.tile([128, DIM], BF, tag="ye", name="yc")
                        nc.sync.dma_start(
                            out=yc[:],
                            in_=_dram_ap(y_d, (8 * g + 2 * tt) * HP * DIM,
                                         [[HP * DIM, 2], [DIM, 64], [1, DIM]]))
                        zt = zts[4 * g + tt]
                        nc.gpsimd.tensor_tensor(out=zt[:], in0=xc[:, tt, :],
                                                in1=yc[:], op=ALU.add)
                        stats = pE1.tile([128, 6], F32, tag="st_e", name="stats")
                        nc.vector.bn_stats(out=stats[:], in_=zt[:])
                        nc.vector.bn_aggr(out=mvall[:, tt, :], in_=stats[:])
                    # rstd / hn deferred to the E2 prologue so phase D's
                    # Exp table is never swapped out

                drr = [0]

                class DGroup:
                    """One attention group; tail stages pipeline into the
                    next group's emission to hide the recip/mult/proj chain."""

                    def __init__(self, gi):
                        self.gi = gi
                        self.p0, self.plen = GROUPS[gi]
                        self.nwin = self.plen // 196
                        self.ets, self.oTs, self.rzs = {}, {}, {}

                    def head(self):
                        gi = self.gi
                        self.kTa = kta2[gi % 3]
                        nc.sync.dma_start(
                            out=self.kTa[0:64, :, 0:self.plen],
                            in_=_dram_ap(kT_d, self.p0,
                                         [[NH * NTOK, 64], [NTOK, NH],
                                          [1, self.plen]]))
                        self.attnT = pDa.tile([128, 4, 416], F8, tag="attnT",
                                              name="attnT")
                        if gi < 2:
                            nc.gpsimd.memset(
                                _ap(self.attnT, 3 * 416, [[1, 392]], p=(0, 128)), 0.0)
                            nc.gpsimd.memset(
                                _ap(self.attnT, 3 * 416, [[1, 392]], p=(0, 1)), 1.0)

                    def stage_a(self, b):
                        nwin, p0 = self.nwin, self.p0
                        for h in (2 * b, 2 * b + 1):
                            # 256-padded so each (i, j) block stays in one bank
                            st = pDs_ps.tile([98, 2, 2, 256], F32, tag="st")
                            for i in range(nwin):
                                for j in range(2):
                                    nc.tensor.matmul(
                                        st[:, i, j, 0:196],
                                        self.kTa[:, h, 196 * i + 98 * j:
                                                 196 * i + 98 * j + 98],
                                        qb[h][:, p0 + 196 * i:p0 + 196 * i + 196],
                                        start=True, stop=True)
                            et = pDet.tile([98, 2, 2, 196], F8, tag="et")
                            if nwin == 2:
                                nc.scalar.activation(out=et[:],
                                                     in_=st[:, :, :, 0:196],
                                                     func=AF.Exp, bias=0.0, scale=1.0)
                            else:
                                nc.scalar.activation(out=et[:, 0, :, :],
                                                     in_=st[:, 0, :, 0:196],
                                                     func=AF.Exp, bias=0.0, scale=1.0)
                            self.ets[h] = et

                    def stage_b(self, b):
                        nwin = self.nwin
                        for h in (2 * b, 2 * b + 1):
                            oT = pDo_ps.tile([VS, 2, 196], F32, tag="oT")
                            for i in range(nwin):
                                nc.tensor.matmul(
                                    oT[:, i, :],
                                    _ap(vw[2 * self.gi + i], h * VS,
                                        [[416, 2], [1, VS]], p=(0, 98)),
                                    self.ets[h][:, i, :, :], start=True, stop=True,
                                    perf_mode=DR)
                            self.oTs[h] = oT

                    def stage_c(self, b):
                        nwin = self.nwin
                        for h in (2 * b, 2 * b + 1):
                            rz = pD.tile([1, 392], F32, tag="rz")
                            nc.vector.reciprocal(out=rz[:, 0:196 * nwin],
                                                 in_=self.oTs[h][64:65, 0:nwin, :])
                            zcb = pD.tile([64, 392], F32, tag="zcb")
                            nc.gpsimd.partition_broadcast(zcb[:, 0:196 * nwin],
                                                          rz[:, 0:196 * nwin])
                            self.rzs[h] = zcb

                    def stage_d(self, b):
                        nwin = self.nwin
                        for h in (2 * b, 2 * b + 1):
                            oT = self.oTs[h]
                            zcb = self.rzs[h]
                            if nwin == 1:
                                dst = _ap(self.attnT, (h // 2) * 416, [[1, 196]],
                                          p=((h % 2) * 64, (h % 2) * 64 + 64))
                                src0 = oT[0:64, 0, :]
                                zsrc = zcb[:, 0:196]
                            else:
                                dst = _ap(self.attnT, (h // 2) * 416,
                                          [[196, 2], [1, 196]],
                                          p=((h % 2) * 64, (h % 2) * 64 + 64))
                                src0 = oT[0:64, :, :]
                                zsrc = _ap(zcb, 0, [[196, 2], [1, 196]], p=(0, 64))
                            nc.vector.tensor_tensor(out=dst, in0=src0,
                                                    in1=zsrc, op=ALU.mult)

                    def body(self):
                        self.stage_a(0); self.stage_b(0); self.stage_c(0)
                        self.stage_a(1); self.stage_d(0); self.stage_b(1)
                        self.stage_c(1)
                        self.stage_a(2); self.stage_d(1); self.stage_b(2)

                    def proj(self):
                        for i in range(self.nwin):
                            w = 2 * self.gi + i
                            wo = (w // 5) * 14 * HP + (w % 5) * 14
                            ysb = pD.tile([98, 2, DIM], BF, tag="ysb")
                            for jj in range(2):
                                pjt = pDm_ps.tile([98, 512], F32, tag="m")
                                pj = pjt[:, 0:DIM]
                                sl = 196 * i + 98 * jj
                                for j in range(2):
                                    nc.tensor.matmul(
                                        pj,
                                        self.attnT[:, 2 * j:2 * j + 2, sl:sl + 98],
                                        wp_t[:, 2 * j:2 * j + 2, :],
                                        start=(j == 0), stop=(j == 1), perf_mode=DR)
                                nc.scalar.activation(out=ysb[:, jj, :], in_=pj,
                                                     func=AF.Identity, bias=0.0,
                                                     scale=FSI)
                                e = [nc.scalar, nc.sync][jj]
                                e.dma_start(
                                    out=_dram_ap(y_d, (wo + 7 * jj * HP) * DIM,
                                                 [[HP * DIM, 7], [DIM, 14],
                                                  [1, DIM]]),
                                    in_=ysb[:, jj, :])

                prev = None
                for gi in range(len(GROUPS)):
                    cur = DGroup(gi)
                    cur.head()
                    cur.stage_a(0)
                    if prev is not None:
                        prev.stage_c(2)
                        prev.stage_d(2)
                    cur.stage_b(0); cur.stage_c(0)
                    if prev is not None:
                        prev.proj()
                        for g in E1_AFTER.get(gi - 1, []):
                            e1_group(g)
                    cur.stage_a(1); cur.stage_d(0); cur.stage_b(1)
                    cur.stage_c(1)
                    cur.stage_a(2); cur.stage_d(1); cur.stage_b(2)
                    prev = cur
                prev.stage_c(2); prev.stage_d(2); prev.proj()
                for g in E1_AFTER.get(len(GROUPS) - 1, []):
                    e1_group(g)

            # ===== Phase E2: fc1 + gelu + fc2 (attention operands freed) =====
            with tc.tile_pool(name="pE2", bufs=3) as pE2, \
                 tc.tile_pool(name="pE2g", bufs=2) as pE2g, \
                 tc.tile_pool(name="pE2h", bufs=2) as pE2h, \
                 tc.tile_pool(name="pE2t_ps", bufs=2, space="PSUM") as pE2t_ps, \
                 tc.tile_pool(name="pE2_ps", bufs=2, space="PSUM") as pE2_ps, \
                 tc.tile_pool(name="pE3_ps", bufs=2, space="PSUM") as pE3_ps:
                def rstd_block(g):
                    yq = pE2.tile([128, 4], F32, tag="yq", name="yq")
                    nc.scalar.activation(out=yq[:], in_=_ap(mvs[g], 1, [[2, 4]]),
                                         func=AF.Sqrt, bias=eps_t[:], scale=1.0)
                    nc.vector.reciprocal(out=yq[:], in_=yq[:])
                    for tt in range(4):
                        nmr = pE2.tile([128, 1], F32, tag="nmr_e", name="nmr")
                        nc.vector.scalar_tensor_tensor(out=nmr[:],
                                                       in0=mvs[g][:, tt, 0:1],
                                                       scalar=-1.0,
                                                       in1=yq[:, tt:tt + 1],
                                                       op0=ALU.mult, op1=ALU.mult)
                        # bf16 SBUF-only: DVE runs this at 4x
                        nc.vector.tensor_scalar(out=hns[4 * g + tt][:],
                                                in0=zts[4 * g + tt][:],
                                                scalar1=nmr[:],
                                                scalar2=yq[:, tt:tt + 1],
                                                op0=ALU.add, op1=ALU.mult)

                # group 7 depends on the very last proj; doing its rstd here
                # would head-of-line-block both Act and DVE queues
                for g in range(7):
                    rstd_block(g)
                nc.scalar.activation(out=dmy_t[:], in_=eps_t[0:1, 0:1],
                                     func=AF.Gelu, bias=0.0, scale=1.0)
                for g in range(8):
                    if g == 7:
                        rstd_block(7)
                    h2T = pE2h.tile([128, 4, 512], F8, tag="h2T", name="h2T")
                    if g < 2:
                        e = [nc.vector, nc.gpsimd][g % 2]
                        e.memset(_ap(h2T, 3 * 512, [[1, 512]], p=(0, 128)), 0.0)
                        e.memset(_ap(h2T, 3 * 512, [[1, 512]], p=(0, 1)), 1.0)
                    for tt in range(4):
                        pt = pE2t_ps.tile([128, 3, 128], BF, tag="htr", name="pt")
                        hn = hns[4 * g + tt]
                        for c in range(3):
                            nc.tensor.transpose(pt[:, c, :], hn[:, c * 128:(c + 1) * 128],
                                                identB[:])
                        dst = _ap(h2T, tt * 128, [[512, 3], [1, 128]])
                        nc.vector.tensor_copy(out=dst, in_=pt[:])
                    gt = [pE2g.tile([128, 2, 512], F8, tag=f"g{p}", name=f"g{p}")
                          for p in range(6)]
                    for p in range(6):
                        ps = pE2_ps.tile([128, 2, 512], F32, tag="fc1", name="ps1")
                        for mh in range(2):
                            m = 2 * p + mh
                            for j in range(2):
                                nc.tensor.matmul(
                                    ps[:, mh, :],
                                    w1_t[:, 2 * j:2 * j + 2, m * 128:(m + 1) * 128],
                                    h2T[:, 2 * j:2 * j + 2, :],
                                    start=(j == 0), stop=(j == 1), perf_mode=DR)
                        nc.scalar.activation(out=gt[p][:], in_=ps[:],
                                             func=AF.Gelu, bias=0.0, scale=FSI)
                    ot = pE2.tile([128, 4, DIM], F32, tag="oe", name="ot")
                    for tt in range(4):
                        ps = pE3_ps.tile([128, DIM], F32, tag="fc2", name="ps2")
                        for p in range(6):
                            nc.tensor.matmul(ps[:], gt[p][:, :, tt * 128:(tt + 1) * 128],
                                             w2_t[:, 2 * p:2 * p + 2, :],
                                             start=(p == 0), stop=False, perf_mode=DR)
                        nc.tensor.matmul(ps[:], ones_f8[:], b2row[:],
                                         start=False, stop=True)
                        nc.vector.scalar_tensor_tensor(out=ot[:, tt, :], in0=ps[:],
                                                       scalar=FSI,
                                                       in1=zts[4 * g + tt][:],
                                                       op0=ALU.mult, op1=ALU.add)
                    nc.sync.dma_start(
                        out=_dram_ap(out_d, 512 * g * DIM,
                                     [[DIM, 128], [128 * DIM, 4], [1, DIM]]),
                        in_=ot[:])

    nc.compile()
    return nc


_NC = None


def _get_nc():
    global _NC
    if _NC is None:
        _NC = build_bass()
    return _NC


def _f8(a):
    return np.ascontiguousarray(
        np.clip(np.asarray(a, np.float32), -240.0, 240.0)).astype(
            ml_dtypes.float8_e4m3)


def _host_prep(inputs):
    f = np.float32
    bf = ml_dtypes.bfloat16
    ln1_w = np.asarray(inputs["ln1_w"], f); ln1_b = np.asarray(inputs["ln1_b"], f)
    qkv_w = np.asarray(inputs["qkv_w"], f); qkv_b = np.asarray(inputs["qkv_b"], f)
    proj_w = np.asarray(inputs["proj_w"], f); proj_b = np.asarray(inputs["proj_b"], f)
    ln2_w = np.asarray(inputs["ln2_w"], f); ln2_b = np.asarray(inputs["ln2_b"], f)
    fc1_w = np.asarray(inputs["fc1_w"], f); fc1_b = np.asarray(inputs["fc1_b"], f)
    fc2_w = np.asarray(inputs["fc2_w"], f); fc2_b = np.asarray(inputs["fc2_b"], f)
    rel_h = np.asarray(inputs["rel_pos_h"], f); rel_w = np.asarray(inputs["rel_pos_w"], f)

    wqk = (ln1_w[:, None] * qkv_w[:, :768]).copy()
    bqk = (ln1_b @ qkv_w[:, :768] + qkv_b[:768]).copy()
    wqk[:, :384] *= SCALE
    bqk[:384] *= SCALE
    wv = (ln1_w[:, None] * qkv_w[:, 768:]).copy()
    bv = ln1_b @ qkv_w[:, 768:] + qkv_b[768:]

    def chunk4(wmat, n, bias_row):
        # [384, n] -> [128, 4, n]: chunks 0..2 = w rows, chunk3 row0 = bias
        out = np.zeros((128, 4, n), f)
        for kc in range(3):
            out[:, kc, :] = wmat[kc * 128:(kc + 1) * 128, :]
        out[0, 3, :] = bias_row
        return out * FS

    wqk4 = chunk4(wqk, 768, np.concatenate([bqk[:384], np.zeros(384, f)]))
    wv4 = chunk4(wv, 384, np.zeros(384, f))
    bp = proj_b + bv @ proj_w
    wp4 = chunk4(proj_w, 384, bp)
    w1m = ln2_w[:, None] * fc1_w
    b1 = ln2_b @ fc1_w + fc1_b
    w14 = chunk4(w1m, MLP, b1)
    w2m = np.zeros((128, 12, DIM), f)
    for kc in range(12):
        w2m[:, kc, :] = fc2_w[kc * 128:(kc + 1) * 128, :]
    w2m *= FS

    coords = np.arange(WS)[:, None] - np.arange(WS)[None, :] + (WS - 1)
    Rh = rel_h[coords]
    Rw = rel_w[coords]
    rel = np.zeros((HD, 2 * 196), f)
    for r in range(14):
        rel[:, r * 14:(r + 1) * 14] = Rh[r].T / SCALE
    for c in range(14):
        rel[:, 196 + c * 14:196 + (c + 1) * 14] = Rw[c].T / SCALE

    kpat = np.zeros((46, 392), f)
    for j in range(14):
        for a in range(2):
            kpat[j, 196 * a + 14 * j:196 * a + 14 * j + 14] = 1.0
            kpat[32 + j, 196 * a + j::14][:14] = 1.0

    return {
        "wqk": _f8(wqk4.reshape(128, -1)),
        "wv": _f8(wv4.reshape(128, -1)),
        "rel": rel.astype(bf),
        "kpat": kpat.astype(bf),
        "wp": _f8(wp4.reshape(128, -1)),
        "w1": _f8(w14.reshape(128, -1)),
        "w2": _f8(w2m.reshape(128, -1)),
        "b2": _f8(fc2_b * FS),
    }


def kernel(**inputs):
    nc = _get_nc()
    shared = _host_prep(inputs)
    x = np.asarray(inputs["x"], np.float32).reshape(B, NVAL, DIM)
    in_maps = [dict(shared, x=np.ascontiguousarray(x[c])) for c in range(B)]
    res = run_bass_kernel_spmd(nc, in_maps, list(range(B)))
    out = np.stack([res.results[c]["out"] for c in range(B)])
    return out.reshape(B, H, W, DIM)


if __name__ == "__main__":
    build_bass()
    print("build ok")


# revision 64
# speedup vs baseline: 1.0005x; 1.0005x over previous
"""Bass/Trainium2 kernel v3 for nn_BlockForNormalWindow (windowed-attention
transformer block), data-parallel over batch across 8 NeuronCores.

v3 over v2: fp8e4 DoubleRow matmuls for qkv/v/proj/fc1/fc2 (weights x64,
biases folded via ones-row in a 4th K-chunk), window-major fp8 hT
(contiguous group slices, no hstage), bf16 score path with K=92 layout
(no garbage rows), fp8 ets/v with DoubleRow AV, PE-broadcast of 1/z,
Quake rsqrt on DVE for LN2 (no act-table thrash), 3-way engine rotation
for PSUM->SBUF copies, split E1/E2 MLP phase."""
import sys
sys.path.insert(0, '/opt/trn_rl_repo')

import numpy as np
import ml_dtypes
import concourse.bass as bass
import concourse.mybir as mybir
import concourse.tile as tile
from concourse import bacc
from concourse.bass_utils import run_bass_kernel_spmd
from concourse.masks import make_identity

F32 = mybir.dt.float32
I32 = mybir.dt.int32
BF = mybir.dt.bfloat16
F8 = mybir.dt.float8e4
AF = mybir.ActivationFunctionType
ALU = mybir.AluOpType
DR = mybir.MatmulPerfMode.DoubleRow

B, H, W = 8, 64, 64
DIM, NH, WS = 384, 6, 14
HD = DIM // NH
MLP = 4 * DIM
EPS = 1e-5
SCALE = HD ** -0.5
HP = 70
NWIN = 25
NTOK = NWIN * WS * WS        # 4900
NVAL = H * W                 # 4096
VS = 65                      # per-head stride in v layout (64 vals + ones col)
KR = 110                     # rows in k/q operand: q/k 0:64, relh 64:78, relw 96:110
FS = 64.0                    # fp8 weight pre-scale
FSI = 1.0 / FS
QMAGIC = 1.3211836172961055e+19   # 0x5f3759df as float32

GROUPS = [(g * 392, 392) for g in range(12)] + [(4704, 196)]
# E1 group g (image rows 8g:8g+8) ready after this D group index
E1_AFTER = {2: [0], 4: [1, 2], 7: [3, 4], 9: [5, 6], 12: [7]}


def _ap(t, offset_elems, dims, p=None):
    a = t[:, 0:1] if p is None else t[p[0]:p[1], 0:1]
    return bass.AP(tensor=a.tensor, offset=a.offset + offset_elems,
                   ap=[a.ap[0]] + dims)


def _dram_ap(t, offset_elems, dims):
    a = t.ap()
    return bass.AP(tensor=a.tensor, offset=offset_elems, ap=dims)


def build_bass():
    nc = bacc.Bacc("TRN2", target_bir_lowering=False, debug=False)

    x_in = nc.dram_tensor("x", [NVAL, DIM], F32, kind="ExternalInput")
    wqk_in = nc.dram_tensor("wqk", [128, 4 * 2 * DIM], F8, kind="ExternalInput")
    wv_in = nc.dram_tensor("wv", [128, 4 * DIM], F8, kind="ExternalInput")
    rel_in = nc.dram_tensor("rel", [HD, 2 * 196], BF, kind="ExternalInput")
    kpat_in = nc.dram_tensor("kpat", [46, 392], BF, kind="ExternalInput")
    wp_in = nc.dram_tensor("wp", [128, 4 * DIM], F8, kind="ExternalInput")
    w1_in = nc.dram_tensor("w1", [128, 4 * MLP], F8, kind="ExternalInput")
    w2_in = nc.dram_tensor("w2", [128, 12 * DIM], F8, kind="ExternalInput")
    b2_in = nc.dram_tensor("b2", [DIM], F8, kind="ExternalInput")
    out_d = nc.dram_tensor("out", [NVAL, DIM], F32, kind="ExternalOutput")

    # k operand in DRAM: rows 0:64 x [NH, NTOK] bf16, written in phase B.
    kT_d = nc.dram_tensor("kT_d", [64, NH * NTOK], BF)
    y_d = nc.dram_tensor("y_d", [HP * HP, DIM], BF)

    with tile.TileContext(nc) as tc:
      with tc.tile_pool(name="singles", bufs=1) as singles:
        ident_f = singles.tile([128, 128], F32)
        make_identity(nc, ident_f[:])
        identB = singles.tile([128, 128], BF)
        nc.vector.tensor_copy(out=identB[:], in_=ident_f[:])

        eps_t = singles.tile([128, 1], F32)
        nc.vector.memset(eps_t[:], EPS)
        cb64 = singles.tile([1, 64], BF)
        nc.gpsimd.memset(cb64[:], 1.0)
        ones_f8 = singles.tile([1, 128], F8)
        nc.gpsimd.memset(ones_f8[:], 1.0)
        onesrow_f = singles.tile([1, 392], F32)
        nc.gpsimd.memset(onesrow_f[:], 1.0)
        magic_t = singles.tile([128, 4], F32)
        nc.vector.memset(magic_t[:], QMAGIC)
        c15_t = singles.tile([128, 4], F32)
        nc.vector.memset(c15_t[:], 1.5)
        dmy_t = singles.tile([1, 1], F32)

        # weights
        wqk_t = singles.tile([128, 4, 2 * DIM], F8)
        nc.sync.dma_start(out=wqk_t[:], in_=wqk_in.ap())
        wv_t = singles.tile([128, 4, DIM], F8)
        nc.sync.dma_start(out=wv_t[:], in_=wv_in.ap())
        relm_t = singles.tile([HD, 2 * 196], BF)
        nc.sync.dma_start(out=relm_t[:], in_=rel_in.ap())
        wp_t = singles.tile([128, 4, DIM], F8)
        nc.sync.dma_start(out=wp_t[:], in_=wp_in.ap())
        w1_t = singles.tile([128, 4, MLP], F8)
        nc.sync.dma_start(out=w1_t[:], in_=w1_in.ap())
        w2_t = singles.tile([128, 12, DIM], F8)
        nc.sync.dma_start(out=w2_t[:], in_=w2_in.ap())
        b2row = singles.tile([1, DIM], F8)
        nc.sync.dma_start(out=b2row[:], in_=b2_in.ap())

        with tc.tile_pool(name="attops", bufs=1) as attops:
          qb = [attops.tile([KR, NTOK], BF, name=f"qb{h}") for h in range(NH)]
          for h in range(NH):
              # zero the pattern-selected rows incl. never-written 78:96 band
              e = [nc.scalar, nc.gpsimd, nc.scalar,
                   nc.vector, nc.scalar, nc.scalar][h]
              if e is nc.scalar:
                  e.memzero(qb[h][64:96, :])
              else:
                  e.memset(qb[h][64:96, :], 0.0)
          vw = [attops.tile([98, 2, 416], F8, name=f"vw{w}") for w in range(NWIN)]
          for w in range(NWIN):
              e = [nc.vector, nc.gpsimd][w % 2]
              e.memset(_ap(vw[w], 64, [[416, 2], [VS, NH], [1, 1]]), 1.0)
          kta2 = [attops.tile([KR, NH, 392], BF, name=f"kta{i}") for i in range(3)]
          for i in range(3):
              nc.sync.dma_start(
                  out=kta2[i][64:KR, :, :],
                  in_=bass.AP(tensor=kpat_in.ap().tensor, offset=0,
                              ap=[[392, 46], [0, NH], [1, 392]]))

          with tc.tile_pool(name="pHT", bufs=1) as pHT:
            # window-block-interleaved: [(w, k, c)] -> w*896 + k*224 + c
            # (DoubleRow ldweights needs 32-elem-aligned k-pair strides)
            hT = pHT.tile([128, 25, 4, 224], F8, name="hT")
            # chunk 3: row0 = 1.0 (bias row), rows 1:128 = 0
            for i in range(4):
                e = [nc.vector, nc.gpsimd][i % 2]
                dims = [[896, 7], [1, 196]]
                o = 3 * 224 + (7 * i) * 896
                if i == 3:
                    dims = [[896, 4], [1, 196]]
                nc.scalar.memzero(_ap(hT, o, dims, p=(0, 128)))
                e.memset(_ap(hT, o, dims, p=(0, 1)), 1.0)
            # zero padding tokens in chunks 0:3 (right-edge and bottom windows)
            for c in range(3):
                e = [nc.vector, nc.gpsimd][c % 2]
                # right-edge windows w%5==4, cols 8:14 of each window row
                e.memset(_ap(hT, 4 * 896 + c * 224 + 8,
                             [[4480, 5], [14, 14], [1, 6]]), 0.0)
                # bottom windows 20..24, rows 8:14
                e.memset(_ap(hT, 20 * 896 + c * 224 + 8 * 14,
                             [[896, 5], [1, 84]]), 0.0)

            # ===== Phase A: LN1 + transpose into window-major fp8 hT =====
            with tc.tile_pool(name="pA", bufs=4) as pA, \
                 tc.tile_pool(name="pA_ps", bufs=4, space="PSUM") as pA_ps:
                for ch in range(8):
                    xc = pA.tile([128, 4, DIM], F32, tag="xc")
                    nc.sync.dma_start(
                        out=xc[:],
                        in_=_dram_ap(x_in, 512 * ch * DIM,
                                     [[DIM, 128], [128 * DIM, 4], [1, DIM]]))
                    mvall = pA.tile([128, 4, 2], F32, tag="mva")
                    for tt in range(4):
                        stats = pA.tile([128, 6], F32, tag="st")
                        nc.vector.bn_stats(out=stats[:], in_=xc[:, tt, :])
                        nc.vector.bn_aggr(out=mvall[:, tt, :], in_=stats[:])
                    rstd = pA.tile([128, 4], F32, tag="rstd")
                    nc.scalar.activation(out=rstd[:], in_=_ap(mvall, 1, [[2, 4]]),
                                         func=AF.Sqrt, bias=eps_t[:], scale=1.0)
                    nc.vector.reciprocal(out=rstd[:], in_=rstd[:])
                    for tt in range(4):
                        t = 4 * ch + tt
                        nmr = pA.tile([128, 1], F32, tag="nmr")
                        nc.vector.scalar_tensor_tensor(out=nmr[:], in0=mvall[:, tt, 0:1],
                                                       scalar=-1.0, in1=rstd[:, tt:tt + 1],
                                                       op0=ALU.mult, op1=ALU.mult)
                        hn = pA.tile([128, DIM], BF, tag="hn")
                        if tt % 2 == 0:
                            nc.gpsimd.tensor_scalar(out=hn[:], in0=xc[:, tt, :],
                                                    scalar1=nmr[:],
                                                    scalar2=rstd[:, tt:tt + 1],
                                                    op0=ALU.add, op1=ALU.mult)
                        else:
                            nc.scalar.activation(out=hn[:], in_=xc[:, tt, :],
                                                 func=AF.Identity, bias=nmr[:],
                                                 scale=rstd[:, tt:tt + 1])
                        pt = pA_ps.tile([128, 3, 128], BF, tag="tr")
                        for c in range(3):
                            nc.tensor.transpose(pt[:, c, :], hn[:, c * 128:(c + 1) * 128],
                                                identB[:])
                        # window-major scatter: rows r0, r0+1 of the image
                        # (4 full col-windows of 14 + 1 partial of 8)
                        r0 = 2 * t
                        band, rb = r0 // 14, r0 % 14
                        for r in range(2):
                            dst = _ap(hT, band * 4480 + (rb + r) * 14,
                                      [[224, 3], [896, 4], [1, 14]])
                            src = bass.AP(tensor=pt[:].tensor,
                                          offset=pt[:].offset + 64 * r,
                                          ap=[pt[:].ap[0], [128, 3], [1, 56]])
                            if (t + r) % 3 != 2:
                                nc.scalar.copy(out=dst, in_=src)
                            else:
                                nc.vector.tensor_copy(out=dst, in_=src)
                            dst = _ap(hT, band * 4480 + 4 * 896 + (rb + r) * 14,
                                      [[224, 3], [1, 8]])
                            src = bass.AP(tensor=pt[:].tensor,
                                          offset=pt[:].offset + 64 * r + 56,
                                          ap=[pt[:].ap[0], [128, 3], [1, 8]])
                            nc.vector.tensor_copy(out=dst, in_=src)

            # ===== Phase B: qk DR matmuls, rel, v =====
            with tc.tile_pool(name="pB", bufs=3) as pB, \
                 tc.tile_pool(name="pB_ps", bufs=2, space="PSUM") as pB_ps, \
                 tc.tile_pool(name="pC_ps", bufs=2, space="PSUM") as pC_ps, \
                 tc.tile_pool(name="pBv_ps", bufs=2, space="PSUM") as pBv_ps:
                copy_rr = [0]

                def scaled_copy(dst, src):
                    i = copy_rr[0]; copy_rr[0] += 1
                    if i % 2 == 0:
                        nc.scalar.activation(out=dst, in_=src, func=AF.Identity,
                                             bias=0.0, scale=FSI)
                    else:
                        nc.vector.tensor_scalar(out=dst, in0=src, scalar1=FSI,
                                                scalar2=None, op0=ALU.mult)

                def emit_v(s_):
                    w, half = s_ // 2, s_ % 2
                    ps = pBv_ps.tile([98, DIM], F32, tag="v", name="vps")
                    for j in range(2):
                        nc.tensor.matmul(ps[:],
                                         _ap(hT, w * 896 + 2 * j * 224 + 98 * half,
                                             [[224, 2], [1, 98]]),
                                         wv_t[:, 2 * j:2 * j + 2, :],
                                         start=(j == 0), stop=(j == 1), perf_mode=DR)
                    scaled_copy(_ap(vw[w], half * 416, [[VS, NH], [1, 64]]), ps[:])

                vnext = [0]
                relc = [0]
                pending_rel = []

                def rel_pack(h, r0, n, is_row):
                    # n rel matmuls into one 4-bank psum tile, one big copy out
                    def go():
                        ps = pC_ps.tile([14, 2, 512], F32, tag="rel")
                        for k in range(n):
                            r = r0 + k
                            if is_row:
                                nc.tensor.matmul(
                                    ps[:, k, 0:350],
                                    relm_t[:, r * 14:(r + 1) * 14],
                                    _ap(qb[h], r * 14, [[196, 25], [1, 14]],
                                        p=(0, 64)),
                                    start=True, stop=True)
                            else:
                                nc.tensor.matmul(
                                    ps[:, k, 0:350],
                                    relm_t[:, 196 + r * 14:196 + (r + 1) * 14],
                                    _ap(qb[h], r, [[196, 25], [14, 14]],
                                        p=(0, 64)),
                                    start=True, stop=True)
                        src = _ap(ps, 0, [[512, n], [1, 350]], p=(0, 14))
                        if is_row:
                            dst = _ap(qb[h], r0 * 14,
                                      [[14, n], [196, 25], [1, 14]], p=(64, 78))
                        else:
                            dst = _ap(qb[h], r0,
                                      [[1, n], [196, 25], [14, 14]], p=(96, 110))
                        i = relc[0]; relc[0] += 1
                        if i % 2 == 0:
                            nc.scalar.copy(out=dst, in_=src)
                        else:
                            nc.vector.tensor_copy(out=dst, in_=src)
                    return go

                for m in range(6):
                    if m == 5:
                        # preload the Exp act table during B's tail so phase
                        # D's first softmax doesn't pay the 1.3us load
                        nc.scalar.activation(out=dmy_t[:], in_=eps_t[0:1, 0:1],
                                             func=AF.Exp, bias=0.0, scale=1.0)
                    for gi, (p0, plen) in enumerate(GROUPS):
                        ps = pB_ps.tile([128, 392], F32, tag="qk")
                        w0 = p0 // 196
                        nwing = plen // 196
                        for j in range(2):
                            nc.tensor.matmul(
                                ps[:, 0:plen],
                                wqk_t[:, 2 * j:2 * j + 2, m * 128:(m + 1) * 128],
                                _ap(hT, w0 * 896 + 2 * j * 224,
                                    [[224, 2], [896, nwing], [1, 196]]),
                                start=(j == 0), stop=(j == 1), perf_mode=DR)
                        if m < 3:
                            for half in range(2):
                                h = 2 * m + half
                                scaled_copy(qb[h][0:64, p0:p0 + plen],
                                            ps[64 * half:64 * half + 64, 0:plen])
                        else:
                            mm = m - 3
                            kst = pB.tile([128, 392], BF, tag="kst", bufs=4,
                                          name="kst")
                            scaled_copy(kst[:, 0:plen], ps[:, 0:plen])
                            nc.sync.dma_start(
                                out=_dram_ap(kT_d, 2 * mm * NTOK + p0,
                                             [[NTOK, 2], [NH * NTOK, 64], [1, plen]]),
                                in_=kst[:, 0:plen])
                        if vnext[0] < 50 and (m, gi) != (0, 0):
                            emit_v(vnext[0])
                            vnext[0] += 1
                        for _ in range(3):
                            if pending_rel:
                                pending_rel.pop(0)()
                    if m < 3:
                        # queue rel rows for heads 2m, 2m+1 (q complete now);
                        # they interleave into the next m's group loop
                        for r0, n in ((0, 2), (2, 2), (4, 2), (6, 2), (8, 2), (10, 2), (12, 2)):
                            for half in range(2):
                                h = 2 * m + half
                                pending_rel.append(rel_pack(h, r0, n, True))
                                pending_rel.append(rel_pack(h, r0, n, False))
                while pending_rel:
                    pending_rel.pop(0)()

          # ===== Phase D: attention + proj (hT freed) — with E1 interleaved ===
          with tc.tile_pool(name="pE1p", bufs=1) as pE1p:
            zts = [pE1p.tile([128, DIM], BF, name=f"zts{t}") for t in range(32)]
            hns = [pE1p.tile([128, DIM], BF, name=f"hns{t}") for t in range(32)]
            mvs = [pE1p.tile([128, 4, 2], F32, name=f"mv{g}") for g in range(8)]

            with tc.tile_pool(name="pD", bufs=4) as pD, \
                 tc.tile_pool(name="pDet", bufs=10) as pDet, \
                 tc.tile_pool(name="pDa", bufs=2) as pDa, \
                 tc.tile_pool(name="pE1", bufs=2) as pE1, \
                 tc.tile_pool(name="pDs_ps", bufs=2, space="PSUM") as pDs_ps, \
                 tc.tile_pool(name="pDo_ps", bufs=2, space="PSUM") as pDo_ps, \
                 tc.tile_pool(name="pDm_ps", bufs=2, space="PSUM") as pDm_ps:

                def e1_group(g):
                    xc = pE1.tile([128, 4, DIM], F32, tag="xe", name="xc")
                    nc.sync.dma_start(
                        out=xc[:],
                        in_=_dram_ap(x_in, 512 * g * DIM,
                                     [[DIM, 128], [128 * DIM, 4], [1, DIM]]))
                    mvall = mvs[g]
                    for tt in range(4):
                        yc = pE1.tile([128, DIM], BF, tag="ye", name="yc")
                        nc.sync.dma_start(
                            out=yc[:],
                            in_=_dram_ap(y_d, (8 * g + 2 * tt) * HP * DIM,
                                         [[HP * DIM, 2], [DIM, 64], [1, DIM]]))
                        zt = zts[4 * g + tt]
                        nc.gpsimd.tensor_tensor(out=zt[:], in0=xc[:, tt, :],
                                                in1=yc[:], op=ALU.add)
                        stats = pE1.tile([128, 6], F32, tag="st_e", name="stats")
                        nc.vector.bn_stats(out=stats[:], in_=zt[:])
                        nc.vector.bn_aggr(out=mvall[:, tt, :], in_=stats[:])
                    # rstd / hn deferred to the E2 prologue so phase D's
                    # Exp table is never swapped out

                drr = [0]

                class DGroup:
                    """One attention group; tail stages pipeline into the
                    next group's emission to hide the recip/mult/proj chain."""

                    def __init__(self, gi):
                        self.gi = gi
                        self.p0, self.plen = GROUPS[gi]
                        self.nwin = self.plen // 196
                        self.ets, self.oTs, self.rzs = {}, {}, {}

                    def head(self):
                        gi = self.gi
                        self.kTa = kta2[gi % 3]
                        nc.sync.dma_start(
                            out=self.kTa[0:64, :, 0:self.plen],
                            in_=_dram_ap(kT_d, self.p0,
                                         [[NH * NTOK, 64], [NTOK, NH],
                                          [1, self.plen]]))
                        self.attnT = pDa.tile([128, 4, 416], F8, tag="attnT",
                                              name="attnT")
                        if gi < 2:
                            nc.gpsimd.memset(
                                _ap(self.attnT, 3 * 416, [[1, 392]], p=(0, 128)), 0.0)
                            nc.gpsimd.memset(
                                _ap(self.attnT, 3 * 416, [[1, 392]], p=(0, 1)), 1.0)

                    def stage_a(self, b):
                        nwin, p0 = self.nwin, self.p0
                        for h in (2 * b, 2 * b + 1):
                            # 256-padded so each (i, j) block stays in one bank
                            st = pDs_ps.tile([98, 2, 2, 256], F32, tag="st")
                            for i in range(nwin):
                                for j in range(2):
                                    nc.tensor.matmul(
                                        st[:, i, j, 0:196],
                                        self.kTa[:, h, 196 * i + 98 * j:
                                                 196 * i + 98 * j + 98],
                                        qb[h][:, p0 + 196 * i:p0 + 196 * i + 196],
                                        start=True, stop=True)
                            et = pDet.tile([98, 2, 2, 196], F8, tag="et")
                            if nwin == 2:
                                nc.scalar.activation(out=et[:],
                                                     in_=st[:, :, :, 0:196],
                                                     func=AF.Exp, bias=0.0, scale=1.0)
                            else:
                                nc.scalar.activation(out=et[:, 0, :, :],
                                                     in_=st[:, 0, :, 0:196],
                                                     func=AF.Exp, bias=0.0, scale=1.0)
                            self.ets[h] = et

                    def stage_b(self, b):
                        nwin = self.nwin
                        for h in (2 * b, 2 * b + 1):
                            oT = pDo_ps.tile([VS, 2, 196], F32, tag="oT")
                            for i in range(nwin):
                                nc.tensor.matmul(
                                    oT[:, i, :],
                                    _ap(vw[2 * self.gi + i], h * VS,
                                        [[416, 2], [1, VS]], p=(0, 98)),
                                    self.ets[h][:, i, :, :], start=True, stop=True,
                                    perf_mode=DR)
                            self.oTs[h] = oT

                    def stage_c(self, b):
                        nwin = self.nwin
                        for h in (2 * b, 2 * b + 1):
                            rz = pD.tile([1, 392], F32, tag="rz")
                            nc.vector.reciprocal(out=rz[:, 0:196 * nwin],
                                                 in_=self.oTs[h][64:65, 0:nwin, :])
                            zcb = pD.tile([64, 392], F32, tag="zcb")
                            nc.gpsimd.partition_broadcast(zcb[:, 0:196 * nwin],
                                                          rz[:, 0:196 * nwin])
                            self.rzs[h] = zcb

                    def stage_d(self, b):
                        nwin = self.nwin
                        for h in (2 * b, 2 * b + 1):
                            oT = self.oTs[h]
                            zcb = self.rzs[h]
                            if nwin == 1:
                                dst = _ap(self.attnT, (h // 2) * 416, [[1, 196]],
                                          p=((h % 2) * 64, (h % 2) * 64 + 64))
                                src0 = oT[0:64, 0, :]
                                zsrc = zcb[:, 0:196]
                            else:
                                dst = _ap(self.attnT, (h // 2) * 416,
                                          [[196, 2], [1, 196]],
                                          p=((h % 2) * 64, (h % 2) * 64 + 64))
                                src0 = oT[0:64, :, :]
                                zsrc = _ap(zcb, 0, [[196, 2], [1, 196]], p=(0, 64))
                            nc.vector.tensor_tensor(out=dst, in0=src0,
                                                    in1=zsrc, op=ALU.mult)

                    def body(self):
                        self.stage_a(0); self.stage_b(0); self.stage_c(0)
                        self.stage_a(1); self.stage_d(0); self.stage_b(1)
                        self.stage_c(1)
                        self.stage_a(2); self.stage_d(1); self.stage_b(2)

                    def proj(self):
                        for i in range(self.nwin):
                            w = 2 * self.gi + i
                            wo = (w // 5) * 14 * HP + (w % 5) * 14
                            ysb = pD.tile([98, 2, DIM], BF, tag="ysb")
                            for jj in range(2):
                                pjt = pDm_ps.tile([98, 512], F32, tag="m")
                                pj = pjt[:, 0:DIM]
                                sl = 196 * i + 98 * jj
                                for j in range(2):
                                    nc.tensor.matmul(
                                        pj,
                                        self.attnT[:, 2 * j:2 * j + 2, sl:sl + 98],
                                        wp_t[:, 2 * j:2 * j + 2, :],
                                        start=(j == 0), stop=(j == 1), perf_mode=DR)
                                nc.scalar.activation(out=ysb[:, jj, :], in_=pj,
                                                     func=AF.Identity, bias=0.0,
                                                     scale=FSI)
                                e = [nc.scalar, nc.sync][jj]
                                e.dma_start(
                                    out=_dram_ap(y_d, (wo + 7 * jj * HP) * DIM,
                                                 [[HP * DIM, 7], [DIM, 14],
                                                  [1, DIM]]),
                                    in_=ysb[:, jj, :])

                prev = None
                for gi in range(len(GROUPS)):
                    cur = DGroup(gi)
                    cur.head()
                    cur.stage_a(0)
                    if prev is not None:
                        prev.stage_c(2)
                        prev.stage_d(2)
                    cur.stage_b(0); cur.stage_c(0)
                    if prev is not None:
                        prev.proj()
                        for g in E1_AFTER.get(gi - 1, []):
                            e1_group(g)
                    cur.stage_a(1); cur.stage_d(0); cur.stage_b(1)
                    cur.stage_c(1)
                    cur.stage_a(2); cur.stage_d(1); cur.stage_b(2)
                    prev = cur
                prev.stage_c(2); prev.stage_d(2); prev.proj()
                for g in E1_AFTER.get(len(GROUPS) - 1, []):
                    e1_group(g)

            # ===== Phase E2: fc1 + gelu + fc2 (attention operands freed) =====
            with tc.tile_pool(name="pE2", bufs=3) as pE2, \
                 tc.tile_pool(name="pE2g", bufs=2) as pE2g, \
                 tc.tile_pool(name="pE2h", bufs=2) as pE2h, \
                 tc.tile_pool(name="pE2t_ps", bufs=2, space="PSUM") as pE2t_ps, \
                 tc.tile_pool(name="pE2_ps", bufs=2, space="PSUM") as pE2_ps, \
                 tc.tile_pool(name="pE3_ps", bufs=2, space="PSUM") as pE3_ps:
                for g in range(8):
                    yq = pE2.tile([128, 4], F32, tag="yq", name="yq")
                    nc.scalar.activation(out=yq[:], in_=_ap(mvs[g], 1, [[2, 4]]),
                                         func=AF.Sqrt, bias=eps_t[:], scale=1.0)
                    nc.vector.reciprocal(out=yq[:], in_=yq[:])
                    for tt in range(4):
                        nmr = pE2.tile([128, 1], F32, tag="nmr_e", name="nmr")
                        nc.vector.scalar_tensor_tensor(out=nmr[:],
                                                       in0=mvs[g][:, tt, 0:1],
                                                       scalar=-1.0,
                                                       in1=yq[:, tt:tt + 1],
                                                       op0=ALU.mult, op1=ALU.mult)
                        # bf16 SBUF-only: DVE runs this at 4x
                        nc.vector.tensor_scalar(out=hns[4 * g + tt][:],
                                                in0=zts[4 * g + tt][:],
                                                scalar1=nmr[:],
                                                scalar2=yq[:, tt:tt + 1],
                                                op0=ALU.add, op1=ALU.mult)
                nc.scalar.activation(out=dmy_t[:], in_=eps_t[0:1, 0:1],
                                     func=AF.Gelu, bias=0.0, scale=1.0)
                for g in range(8):
                    h2T = pE2h.tile([128, 4, 512], F8, tag="h2T", name="h2T")
                    if g < 2:
                        e = [nc.vector, nc.gpsimd][g % 2]
                        e.memset(_ap(h2T, 3 * 512, [[1, 512]], p=(0, 128)), 0.0)
                        e.memset(_ap(h2T, 3 * 512, [[1, 512]], p=(0, 1)), 1.0)
                    for tt in range(4):
                        pt = pE2t_ps.tile([128, 3, 128], BF, tag="htr", name="pt")
                        hn = hns[4 * g + tt]
                        for c in range(3):
                            nc.tensor.transpose(pt[:, c, :], hn[:, c * 128:(c + 1) * 128],
                                                identB[:])
                        dst = _ap(h2T, tt * 128, [[512, 3], [1, 128]])
                        nc.vector.tensor_copy(out=dst, in_=pt[:])
                    gt = [pE2g.tile([128, 2, 512], F8, tag=f"g{p}", name=f"g{p}")
                          for p in range(6)]
                    for p in range(6):
                        ps = pE2_ps.tile([128, 2, 512], F32, tag="fc1", name="ps1")
                        for mh in range(2):
                            m = 2 * p + mh
                            for j in range(2):
                                nc.tensor.matmul(
                                    ps[:, mh, :],
                                    w1_t[:, 2 * j:2 * j + 2, m * 128:(m + 1) * 128],
                                    h2T[:, 2 * j:2 * j + 2, :],
                                    start=(j == 0), stop=(j == 1), perf_mode=DR)
                        nc.scalar.activation(out=gt[p][:], in_=ps[:],
                                             func=AF.Gelu, bias=0.0, scale=FSI)
                    ot = pE2.tile([128, 4, DIM], F32, tag="oe", name="ot")
                    for tt in range(4):
                        ps = pE3_ps.tile([128, DIM], F32, tag="fc2", name="ps2")
                        for p in range(6):
                            nc.tensor.matmul(ps[:], gt[p][:, :, tt * 128:(tt + 1) * 128],
                                             w2_t[:, 2 * p:2 * p + 2, :],
                                             start=(p == 0), stop=False, perf_mode=DR)
                        nc.tensor.matmul(ps[:], ones_f8[:], b2row[:],
                                         start=False, stop=True)
                        nc.vector.scalar_tensor_tensor(out=ot[:, tt, :], in0=ps[:],
                                                       scalar=FSI,
                                                       in1=zts[4 * g + tt][:],
                                                       op0=ALU.mult, op1=ALU.add)
                    nc.sync.dma_start(
                        out=_dram_ap(out_d, 512 * g * DIM,
                                     [[DIM, 128], [128 * DIM, 4], [1, DIM]]),
                        in_=ot[:])

    nc.compile()
    return nc


_NC = None


def _get_nc():
    global _NC
    if _NC is None:
        _NC = build_bass()
    return _NC


def _f8(a):
    return np.ascontiguousarray(
        np.clip(np.asarray(a, np.float32), -240.0, 240.0)).astype(
            ml_dtypes.float8_e4m3)


def _host_prep(inputs):
    f = np.float32
    bf = ml_dtypes.bfloat16
    ln1_w = np.asarray(inputs["ln1_w"], f); ln1_b = np.asarray(inputs["ln1_b"], f)
    qkv_w = np.asarray(inputs["qkv_w"], f); qkv_b = np.asarray(inputs["qkv_b"], f)
    proj_w = np.asarray(inputs["proj_w"], f); proj_b = np.asarray(inputs["proj_b"], f)
    ln2_w = np.asarray(inputs["ln2_w"], f); ln2_b = np.asarray(inputs["ln2_b"], f)
    fc1_w = np.asarray(inputs["fc1_w"], f); fc1_b = np.asarray(inputs["fc1_b"], f)
    fc2_w = np.asarray(inputs["fc2_w"], f); fc2_b = np.asarray(inputs["fc2_b"], f)
    rel_h = np.asarray(inputs["rel_pos_h"], f); rel_w = np.asarray(inputs["rel_pos_w"], f)

    wqk = (ln1_w[:, None] * qkv_w[:, :768]).copy()
    bqk = (ln1_b @ qkv_w[:, :768] + qkv_b[:768]).copy()
    wqk[:, :384] *= SCALE
    bqk[:384] *= SCALE
    wv = (ln1_w[:, None] * qkv_w[:, 768:]).copy()
    bv = ln1_b @ qkv_w[:, 768:] + qkv_b[768:]

    def chunk4(wmat, n, bias_row):
        # [384, n] -> [128, 4, n]: chunks 0..2 = w rows, chunk3 row0 = bias
        out = np.zeros((128, 4, n), f)
        for kc in range(3):
            out[:, kc, :] = wmat[kc * 128:(kc + 1) * 128, :]
        out[0, 3, :] = bias_row
        return out * FS

    wqk4 = chunk4(wqk, 768, np.concatenate([bqk[:384], np.zeros(384, f)]))
    wv4 = chunk4(wv, 384, np.zeros(384, f))
    bp = proj_b + bv @ proj_w
    wp4 = chunk4(proj_w, 384, bp)
    w1m = ln2_w[:, None] * fc1_w
    b1 = ln2_b @ fc1_w + fc1_b
    w14 = chunk4(w1m, MLP, b1)
    w2m = np.zeros((128, 12, DIM), f)
    for kc in range(12):
        w2m[:, kc, :] = fc2_w[kc * 128:(kc + 1) * 128, :]
    w2m *= FS

    coords = np.arange(WS)[:, None] - np.arange(WS)[None, :] + (WS - 1)
    Rh = rel_h[coords]
    Rw = rel_w[coords]
    rel = np.zeros((HD, 2 * 196), f)
    for r in range(14):
        rel[:, r * 14:(r + 1) * 14] = Rh[r].T / SCALE
    for c in range(14):
        rel[:, 196 + c * 14:196 + (c + 1) * 14] = Rw[c].T / SCALE

    kpat = np.zeros((46, 392), f)
    for j in range(14):
        for a in range(2):
            kpat[j, 196 * a + 14 * j:196 * a + 14 * j + 14] = 1.0
            kpat[32 + j, 196 * a + j::14][:14] = 1.0

    return {
        "wqk": _f8(wqk4.reshape(128, -1)),
        "wv": _f8(wv4.reshape(128, -1)),
        "rel": rel.astype(bf),
        "kpat": kpat.astype(bf),
        "wp": _f8(wp4.reshape(128, -1)),
        "w1": _f8(w14.reshape(128, -1)),
        "w2": _f8(w2m.reshape(128, -1)),
        "b2": _f8(fc2_b * FS),
    }


def kernel(**inputs):
    nc = _get_nc()
    shared = _host_prep(inputs)
    x = np.asarray(inputs["x"], np.float32).reshape(B, NVAL, DIM)
    in_maps = [dict(shared, x=np.ascontiguousarray(x[c])) for c in range(B)]
    res = run_bass_kernel_spmd(nc, in_maps, list(range(B)))
    out = np.stack([res.results[c]["out"] for c in range(B)])
    return out.reshape(B, H, W, DIM)


if __name__ == "__main__":
    build_bass()
    print("build ok")
